# revision 1
# baseline (speedup 1.0000x reference)
"""Bass/Trainium2 kernel for BestMatchDistance.

ref: sim[b,q,s] = sum_d q[b,d,q]*s[b,d,s]; out[b] = mean_q max_s sim.

Sharding: batch dim B=64 split across 8 cores (8 batches/core), pure data
parallel. Inputs are cast to bf16 on the host (full-rate PE, half DMA).

Per (batch, 128-query tile): the [128, 2048] sim row = 4 bf16 matmuls
(K=64, N=512) K-packed 2-up onto PE row-groups 0-63 / 64-127 (query data is
duplicated to both partition halves, support is split), so weight loads and
matmuls of the two groups overlap on the systolic array. The s-columns land
permuted across PSUM, which is irrelevant under a max-reduce.

PSUM per row: A=[128,512] (1 bank) + B=[128,1536] (3 banks), double
buffered = 8 banks. Evacuation is split across the only two engines with
PSUM read ports (concurrent PE-write + VectorE-read of PSUM serializes on
HW, so most of the row goes through ScalarE):
  - VectorE reduce_max on the single A bank
  - ScalarE copies B to SBUF as bf16; VectorE max-reduces the copies with a
    bf16 tensor_tensor tree (2 elem/cycle), batched 16 rows/instruction.
Mean over queries = free-dim reduce_sum + ones-vector matmul over
partitions, scaled by 1/NQ.
"""

import numpy as np

B, D, NQ, NS = 64, 64, 2048, 2048
XW_CFG = 512  # PSUM A width (direct DVE reduce); rest goes via ACT copy
TREE_HB = 16  # rows per bf16-tree instruction batch
TREE_MIN = 96  # smallest TT level width; tail reduce runs on this width
B_FIRST = True  # emit B (ACT-copied) matmuls before the A (DVE) matmul
N_CORES = 8
BPC = B // N_CORES  # batches per core

_cache = {}


def _emit_body(nc, mybir, q_d, s_d, o_d, ones, rall, pools, rep=0, parts=31):
    DO_MM = parts & 1
    DO_RA = parts & 2
    DO_CP = parts & 4
    DO_TREE = parts & 8
    f32 = mybir.dt.float32
    bf16 = mybir.dt.bfloat16
    fmax = mybir.AluOpType.max
    X = mybir.AxisListType.X
    qp, sp, pa, pb, bcp, trp, rp, finp = pools

    n_qt = NQ // 128  # 16 q-tiles per batch
    HB = TREE_HB  # rows per tree batch
    XW = XW_CFG  # direct-reduce width (PSUM A)
    YW = NS - XW  # ACT-copied width (PSUM B), 2 banks
    HNS = NS // 2  # support cols per row-group

    for b in range(BPC):
        qt = qp.tile([128, NQ], bf16, tag="q", name=f"q{rep}_{b}")
        nc.sync.dma_start(out=qt[0:64, :], in_=q_d[b])
        nc.sync.dma_start(out=qt[64:128, :], in_=q_d[b])
        st = sp.tile([128, HNS], bf16, tag="s", name=f"s{rep}_{b}")
        nc.sync.dma_start(out=st[0:64, :], in_=s_d[b][:, 0:HNS])
        nc.sync.dma_start(out=st[64:128, :], in_=s_d[b][:, HNS:NS])

        rA = None
        if XW > 0:
            rA = rp.tile(
                [128, n_qt, XW // 512], f32, tag="rA", name=f"rA{rep}_{b}"
            )
            if not DO_RA:
                nc.vector.memset(rA[:], 0.0)
        rB = rp.tile([128, n_qt], bf16, tag="rB", name=f"rB{rep}_{b}")
        if not (DO_TREE and DO_CP):
            nc.vector.memset(rB[:], 0.0)

        for h in range(n_qt // HB):
            bc = bcp.tile([128, HB, YW], bf16, tag="bc", name=f"bc{rep}_{b}_{h}")
            for r in range(HB):
                i = h * HB + r
                A = (
                    pa.tile([128, XW], f32, tag="A", name=f"A{rep}_{b}_{i}")
                    if XW > 0
                    else None
                )
                Bt = pb.tile([128, YW], f32, tag="B", name=f"B{rep}_{b}_{i}")
                lhs0 = qt[0:64, i * 128 : (i + 1) * 128]
                lhs1 = qt[64:128, i * 128 : (i + 1) * 128]
                if DO_MM:
                    # 4 N=512 matmuls, K-packed: grp0 covers s-cols [0,HNS),
                    # grp1 covers [HNS,NS). Destinations fill A banks then B.
                    dsts = [
                        (A, j * 512) for j in range(XW // 512)
                    ] + [(Bt, j * 512) for j in range(YW // 512)]
                    if B_FIRST:
                        dsts = dsts[XW // 512 :] + dsts[: XW // 512]
                    for k4 in range(4):
                        grp = k4 % 2
                        sc = (k4 // 2) * 512
                        dst, off = dsts[k4]
                        if grp == 0:
                            nc.tensor.matmul(
                                dst[:, off : off + 512], lhsT=lhs0,
                                rhs=st[0:64, sc : sc + 512],
                                start=True, stop=True,
                            )
                        else:
                            nc.tensor.matmul(
                                dst[:, off : off + 512], lhsT=lhs1,
                                rhs=st[64:128, sc : sc + 512],
                                start=True, stop=True, tile_position=(64, 0),
                            )
                if DO_RA:
                    for j in range(XW // 512):
                        nc.vector.reduce_max(
                            rA[:, i, j : j + 1],
                            A[:, j * 512 : (j + 1) * 512],
                            axis=X,
                        )
                if DO_CP:
                    nc.scalar.copy(out=bc[:, r], in_=Bt[:])

            if not (DO_TREE and DO_CP):
                continue
            # bf16 max tree over [128, HB, YW] -> [128, HB]
            cur_t = bc
            w = YW // 2
            lvl = 0
            while w >= TREE_MIN:
                nxt_t = trp.tile(
                    [128, HB, w], bf16, tag=f"t{lvl}", name=f"t{lvl}_{rep}_{b}_{h}"
                )
                nc.vector.tensor_tensor(
                    out=nxt_t[:], in0=cur_t[:, :, 0:w],
                    in1=cur_t[:, :, w : 2 * w], op=fmax,
                )
                cur_t = nxt_t
                w //= 2
                lvl += 1
            nc.vector.reduce_max(rB[:, h * HB : (h + 1) * HB], cur_t[:], axis=X)

        # combine: per-q max over {A bank maxes, B tree maxes}
        nb = XW // 512
        cur = rB
        for j in range(nb):
            nxt = rp.tile(
                [128, n_qt], f32, tag=f"rc{j}", name=f"rc{j}_{rep}_{b}"
            )
            nc.vector.tensor_tensor(
                out=nxt[:], in0=cur[:], in1=rA[:, :, j], op=fmax
            )
            cur = nxt
        nc.vector.reduce_sum(rall[:, b : b + 1], cur[:], axis=X)

    if XW == 0:
        pf = pb.tile([1, BPC], f32, tag="B", name=f"pf{rep}")
    else:
        pf = pa.tile([1, BPC], f32, tag="A", name=f"pf{rep}")
    nc.tensor.matmul(pf[:], lhsT=ones[:], rhs=rall[:], start=True, stop=True)
    ob = finp.tile([1, BPC], f32, tag="ob", name=f"ob{rep}")
    nc.scalar.mul(ob[:], pf[:], 1.0 / NQ)
    nc.sync.dma_start(out=o_d[:], in_=ob[:])


def _build(loop_reps=None, parts=31):
    import concourse.bacc as bacc
    import concourse.mybir as mybir
    import concourse.tile as tile

    f32 = mybir.dt.float32
    bf16 = mybir.dt.bfloat16

    nc = bacc.Bacc("TRN2", target_bir_lowering=False, debug=False)
    q_d = nc.dram_tensor("q", [BPC, D, NQ], bf16, kind="ExternalInput").ap()
    s_d = nc.dram_tensor("s", [BPC, D, NS], bf16, kind="ExternalInput").ap()
    o_d = nc.dram_tensor("o", [1, BPC], f32, kind="ExternalOutput").ap()

    with tile.TileContext(nc) as tc:
        with (
            tc.tile_pool(name="qp", bufs=3) as qp,
            tc.tile_pool(name="sp", bufs=3) as sp,
            tc.tile_pool(name="pa", bufs=2, space="PSUM") as pa,
            tc.tile_pool(name="pb", bufs=2, space="PSUM") as pb,
            tc.tile_pool(name="bcp", bufs=2) as bcp,
            tc.tile_pool(name="tree", bufs=2) as trp,
            tc.tile_pool(name="rp", bufs=2) as rp,
            tc.tile_pool(name="fin", bufs=1) as finp,
        ):
            ones = finp.tile([128, 1], f32, tag="ones")
            nc.vector.memset(ones[:], 1.0)
            rall = finp.tile([128, BPC], f32, tag="rall")
            pools = (qp, sp, pa, pb, bcp, trp, rp, finp)

            if loop_reps is None:
                _emit_body(nc, mybir, q_d, s_d, o_d, ones, rall, pools, parts=parts)
            else:
                with tc.For_i(0, loop_reps, 1):
                    _emit_body(
                        nc, mybir, q_d, s_d, o_d, ones, rall, pools, parts=parts
                    )

    nc.compile()
    return nc


def _to_bf16(x):
    import ml_dtypes

    return np.ascontiguousarray(x, dtype=np.float32).astype(ml_dtypes.bfloat16)


def kernel(query_local, support_local):
    from concourse.bass_utils import run_bass_kernel_spmd

    if "nc" not in _cache:
        _cache["nc"] = _build()
    nc = _cache["nc"]

    q = _to_bf16(query_local).reshape(N_CORES, BPC, D, NQ)
    s = _to_bf16(support_local).reshape(N_CORES, BPC, D, NS)
    in_maps = [{"q": q[c], "s": s[c]} for c in range(N_CORES)]
    res = run_bass_kernel_spmd(nc, in_maps, list(range(N_CORES)))
    outs = [np.asarray(res.results[c]["o"]).reshape(BPC) for c in range(N_CORES)]
    return np.concatenate(outs, axis=0)



# revision 10
# speedup vs baseline: 3.0995x; 3.0995x over previous
"""Bass/Trainium2 kernel for BestMatchDistance.

ref: sim[b,q,s] = sum_d q[b,d,q]*s[b,d,s]; out[b] = mean_q max_s sim.

Sharding: batch dim B=64 split across 8 cores (8 batches/core), pure data
parallel. Inputs are cast to bf16 on the host (full-rate PE, half DMA).

Per (batch, 128-query tile) the [128, 2048] sim row is reduced to a
per-query max. Only VectorE (DVE) and ScalarE (ACT) can read PSUM, a DVE
instruction may read at most one PSUM operand, and every DVE op pays a
pipe-drain roughly equal to its own duration, so the kernel balances two
evacuation paths across the 16 q-tiles of each batch (all matmuls run in
64x64 PE-tiling mode; an accumulation pair must stay on one PE sub-tile,
which is why s_b / s_diff are duplicated to both partition halves):

- v2 path (ACT-light / DVE-heavy): sims of the two support halves land in
  PSUM A/B via sub-tiles T0/T2/T8/T10; ACT copies B to SBUF bf16; one DVE
  tensor_tensor_scan (running max, stride-0 output AP) folds {A, copy(B)}
  into the row max (1024 positions).

- pe-max path (ACT-heavy / DVE-light): the host also ships s_b and
  s_diff = s_a - s_b. PE computes m = q*s_b on T0/T10 (accumulation group
  left open) and d = q*s_diff on T8/T2; ACT computes relu(d) -> SBUF;
  identity matmuls on the SAME sub-tiles T0/T10 accumulate relu(d) onto m,
  i.e. m = max(a_sim, b_sim) elementwise (max(a,b) = b + relu(a-b)); ACT
  copies half of m out; DVE scans only 512 positions.

Mean over queries = free-dim reduce_sum, then a 64x64-mode ones-matmul per
partition half + ACT/DVE combine, scaled by 1/NQ.
"""

import numpy as np

B, D, NQ, NS = 64, 64, 2048, 2048
N_CORES = 8
BPC = B // N_CORES  # batches per core
HNS = NS // 2  # support cols per half
PMN = 9  # pe-max-path tiles per 16 q-tiles (rest take the v2 path)

_cache = {}


def _pm_flags(n_qt, pmn):
    # Bresenham spread of pmn pe-max tiles across n_qt tile slots.
    return [((i + 1) * pmn) // n_qt - (i * pmn) // n_qt > 0 for i in range(n_qt)]


def _emit_body(nc, mybir, q_d, s_d, t_d, o_d, id64, ones, rall, pools, rep=0,
               parts=3, pmn=PMN):
    DO_MM = parts & 1
    DO_EVAC = parts & 2
    f32 = mybir.dt.float32
    bf16 = mybir.dt.bfloat16
    fmax = mybir.AluOpType.max
    fadd = mybir.AluOpType.add
    Relu = mybir.ActivationFunctionType.Relu
    X = mybir.AxisListType.X
    qp, sp, tp, pa, pb, scp, rp, finp = pools

    n_qt = NQ // 128  # 16 q-tiles per batch
    flags = _pm_flags(n_qt, pmn)

    for b in range(BPC):
        qt = qp.tile([128, NQ], bf16, tag="q", name=f"q{rep}_{b}")
        nc.sync.dma_start(out=qt[0:64, :], in_=q_d[b])
        nc.sync.dma_start(out=qt[64:128, :], in_=q_d[b])
        st = sp.tile([128, HNS], bf16, tag="s", name=f"s{rep}_{b}")
        nc.sync.dma_start(out=st[0:64, :], in_=s_d[b][:, 0:HNS])
        nc.sync.dma_start(out=st[64:128, :], in_=s_d[b][:, HNS:NS])
        stb = tp.tile([128, HNS], bf16, tag="tb", name=f"tb{rep}_{b}")
        nc.sync.dma_start(out=stb[0:64, :], in_=t_d[b][:, 0:HNS])
        nc.sync.dma_start(out=stb[64:128, :], in_=t_d[b][:, 0:HNS])
        std = tp.tile([128, HNS], bf16, tag="td", name=f"td{rep}_{b}")
        nc.sync.dma_start(out=std[0:64, :], in_=t_d[b][:, HNS:NS])
        nc.sync.dma_start(out=std[64:128, :], in_=t_d[b][:, HNS:NS])

        pmax = rp.tile([128, n_qt], f32, tag="pmax", name=f"pm{rep}_{b}")
        if not (DO_MM and DO_EVAC):
            nc.vector.memset(pmax[:], 0.0)

        for i in range(n_qt):
            qb = i * 128
            lo = qt[0:64, qb : qb + 64]
            hi_lo = qt[64:128, qb : qb + 64]
            lo_hi = qt[0:64, qb + 64 : qb + 128]
            hi = qt[64:128, qb + 64 : qb + 128]
            A = pa.tile([128, HNS], f32, tag="pa", name=f"A{rep}_{b}_{i}")
            Bt = pb.tile([128, HNS], f32, tag="pb", name=f"B{rep}_{b}_{i}")
            if not DO_MM:
                continue
            if not flags[i]:
                # ---- v2 path: A = a-half sims, Bt = b-half sims ----
                for j in range(2):
                    sl = slice(j * 512, (j + 1) * 512)
                    nc.tensor.matmul(A[0:64, sl], lhsT=lo, rhs=st[0:64, sl],
                                     start=True, stop=True, tile_position=(0, 0))
                    nc.tensor.matmul(A[64:128, sl], lhsT=lo_hi, rhs=st[0:64, sl],
                                     start=True, stop=True, tile_position=(0, 64))
                    nc.tensor.matmul(Bt[0:64, sl], lhsT=hi_lo, rhs=st[64:128, sl],
                                     start=True, stop=True, tile_position=(64, 0))
                    nc.tensor.matmul(Bt[64:128, sl], lhsT=hi, rhs=st[64:128, sl],
                                     start=True, stop=True, tile_position=(64, 64))
                if DO_EVAC:
                    bh = scp.tile([128, HNS], bf16, tag="bh",
                                  name=f"bh{rep}_{b}_{i}")
                    nc.scalar.copy(out=bh[:], in_=Bt[:])
                    nc.vector.tensor_tensor_scan(
                        out=pmax[:, i : i + 1].broadcast_to([128, HNS]),
                        data0=A[:], data1=bh[:], initial=-1e30,
                        op0=fmax, op1=fmax)
            else:
                # ---- pe-max path: A = m (group open), Bt = d ----
                for j in range(2):
                    sl = slice(j * 512, (j + 1) * 512)
                    nc.tensor.matmul(A[0:64, sl], lhsT=lo, rhs=stb[0:64, sl],
                                     start=True, stop=False, tile_position=(0, 0))
                    nc.tensor.matmul(A[64:128, sl], lhsT=hi, rhs=stb[64:128, sl],
                                     start=True, stop=False, tile_position=(64, 64))
                    nc.tensor.matmul(Bt[0:64, sl], lhsT=hi_lo, rhs=std[64:128, sl],
                                     start=True, stop=True, tile_position=(64, 0))
                    nc.tensor.matmul(Bt[64:128, sl], lhsT=lo_hi, rhs=std[0:64, sl],
                                     start=True, stop=True, tile_position=(0, 64))
                rl = scp.tile([128, HNS], bf16, tag="rl", name=f"rl{rep}_{b}_{i}")
                nc.scalar.activation(out=rl[:], in_=Bt[:], func=Relu)
                for j in range(2):
                    sl = slice(j * 512, (j + 1) * 512)
                    nc.tensor.matmul(A[0:64, sl], lhsT=id64[0:64, :],
                                     rhs=rl[0:64, sl], start=False, stop=True,
                                     tile_position=(0, 0))
                    nc.tensor.matmul(A[64:128, sl], lhsT=id64[64:128, :],
                                     rhs=rl[64:128, sl], start=False, stop=True,
                                     tile_position=(64, 64))
                if DO_EVAC:
                    mh = scp.tile([128, 512], bf16, tag="mh",
                                  name=f"mh{rep}_{b}_{i}")
                    nc.scalar.copy(out=mh[:], in_=A[:, 512:1024])
                    nc.vector.tensor_tensor_scan(
                        out=pmax[:, i : i + 1].broadcast_to([128, 512]),
                        data0=A[:, 0:512], data1=mh[:], initial=-1e30,
                        op0=fmax, op1=fmax)

        nc.vector.reduce_sum(rall[:, b : b + 1], pmax[:], axis=X)

    # partition-sum of rall via two 64x64-mode ones-matmuls (separate banks),
    # then combine on ACT+DVE (a K=128 matmul would force a PE mode switch).
    pf1 = pa.tile([128, HNS], f32, tag="pa", name=f"pf1{rep}")
    pf2 = pb.tile([128, HNS], f32, tag="pb", name=f"pf2{rep}")
    nc.tensor.matmul(pf1[0:1, 0:BPC], lhsT=ones[0:64, :], rhs=rall[0:64, :],
                     start=True, stop=True, tile_position=(0, 0))
    nc.tensor.matmul(pf2[0:1, 0:BPC], lhsT=ones[64:128, :], rhs=rall[64:128, :],
                     start=True, stop=True, tile_position=(64, 0))
    h2 = finp.tile([1, BPC], f32, tag="h2", name=f"h2{rep}")
    nc.scalar.copy(out=h2[:], in_=pf2[0:1, 0:BPC])
    hs = finp.tile([1, BPC], f32, tag="hs", name=f"hs{rep}")
    nc.vector.tensor_tensor(out=hs[:], in0=pf1[0:1, 0:BPC], in1=h2[:], op=fadd)
    ob = finp.tile([1, BPC], f32, tag="ob", name=f"ob{rep}")
    nc.scalar.mul(ob[:], hs[:], 1.0 / NQ)
    nc.sync.dma_start(out=o_d[:], in_=ob[:])


def _build(loop_reps=None, parts=3, pmn=PMN):
    import concourse.bacc as bacc
    import concourse.mybir as mybir
    import concourse.tile as tile

    f32 = mybir.dt.float32
    bf16 = mybir.dt.bfloat16
    NE = mybir.AluOpType.not_equal

    nc = bacc.Bacc("TRN2", target_bir_lowering=False, debug=False)
    q_d = nc.dram_tensor("q", [BPC, D, NQ], bf16, kind="ExternalInput").ap()
    s_d = nc.dram_tensor("s", [BPC, D, NS], bf16, kind="ExternalInput").ap()
    t_d = nc.dram_tensor("t", [BPC, D, NS], bf16, kind="ExternalInput").ap()
    o_d = nc.dram_tensor("o", [1, BPC], f32, kind="ExternalOutput").ap()

    with tile.TileContext(nc) as tc:
        with (
            tc.tile_pool(name="qp", bufs=3) as qp,
            tc.tile_pool(name="sp", bufs=3) as sp,
            tc.tile_pool(name="tp", bufs=3) as tp,
            tc.tile_pool(name="pa", bufs=2, space="PSUM") as pa,
            tc.tile_pool(name="pb", bufs=2, space="PSUM") as pb,
            tc.tile_pool(name="scp", bufs=2) as scp,
            tc.tile_pool(name="rp", bufs=2) as rp,
            tc.tile_pool(name="fin", bufs=1) as finp,
        ):
            ones = finp.tile([128, 1], f32, tag="ones")
            nc.vector.memset(ones[:], 1.0)
            # id64: identity-64 in each partition half (for T0/T10 I-matmuls)
            id64 = finp.tile([128, 64], bf16, tag="id64")
            nc.gpsimd.memset(id64[:], 0.0)
            nc.gpsimd.affine_select(out=id64[:], in_=id64[:], compare_op=NE,
                                    fill=1.0, base=0, pattern=[[-1, 64]],
                                    channel_multiplier=1)
            nc.gpsimd.affine_select(out=id64[:], in_=id64[:], compare_op=NE,
                                    fill=1.0, base=-64, pattern=[[-1, 64]],
                                    channel_multiplier=1)
            rall = finp.tile([128, BPC], f32, tag="rall")
            pools = (qp, sp, tp, pa, pb, scp, rp, finp)

            if loop_reps is None:
                _emit_body(nc, mybir, q_d, s_d, t_d, o_d, id64, ones, rall,
                           pools, parts=parts, pmn=pmn)
            else:
                with tc.For_i(0, loop_reps, 1):
                    _emit_body(nc, mybir, q_d, s_d, t_d, o_d, id64, ones,
                               rall, pools, parts=parts, pmn=pmn)

    nc.compile()
    return nc


def _to_bf16(x):
    import ml_dtypes

    return np.ascontiguousarray(x, dtype=np.float32).astype(ml_dtypes.bfloat16)


def _prep(query_local, support_local):
    import ml_dtypes

    q = _to_bf16(query_local).reshape(N_CORES, BPC, D, NQ)
    s = _to_bf16(support_local).reshape(N_CORES, BPC, D, NS)
    sf = np.asarray(s, dtype=np.float32)
    sa, sb = sf[..., 0:HNS], sf[..., HNS:NS]
    # pe-max layout: cols 0:HNS = s_b, cols HNS:NS = s_a - s_b
    t = np.concatenate([sb, sa - sb], axis=-1).astype(ml_dtypes.bfloat16)
    return q, s, t


def kernel(query_local, support_local):
    from concourse.bass_utils import run_bass_kernel_spmd

    if "nc" not in _cache:
        _cache["nc"] = _build()
    nc = _cache["nc"]

    q, s, t = _prep(query_local, support_local)
    in_maps = [{"q": q[c], "s": s[c], "t": t[c]} for c in range(N_CORES)]
    res = run_bass_kernel_spmd(nc, in_maps, list(range(N_CORES)))
    outs = [np.asarray(res.results[c]["o"]).reshape(BPC) for c in range(N_CORES)]
    return np.concatenate(outs, axis=0)


# revision 14
# speedup vs baseline: 4.5098x; 1.4550x over previous
"""Bass/Trainium2 kernel for BestMatchDistance.

ref: sim[b,q,s] = sum_d q[b,d,q]*s[b,d,s]; out[b] = mean_q max_s sim.

Sharding: batch dim B=64 split across 8 cores (8 batches/core), pure data
parallel. Inputs are cast to bf16 on the host (full-rate PE, half DMA).

Per (batch, 128-query tile): the [128, 2048] sim row = 4 bf16 matmuls
(K=64, N=512) K-packed 2-up onto PE row-groups 0-63 / 64-127 (query data is
duplicated to both partition halves, support is split), so the two groups'
matmuls run concurrently. a-half sims land in PSUM tile A, b-half sims in
PSUM tile B (separate pools; B is consumed early by ScalarE so it gets 1
buffer, A is held until the scan so it gets 3 — deepening the pipeline).

Evacuation: ScalarE copies B to SBUF bf16 (no drain penalty on ACT); the
DVE folds {A, copy(B)} with running-max tensor_tensor_scans whose stride-0
output APs leave each final state in pmax (one 1024-position scan per
q-tile; finer splits measured slower on HW — per-op overhead exceeds the
drain savings).

Mean over queries = reduce_max over the SPLIT partials, reduce_sum over
tiles, ones-vector matmul over partitions, scaled by 1/NQ.
"""

import numpy as np

B, D, NQ, NS = 64, 64, 2048, 2048
N_CORES = 8
BPC = B // N_CORES  # batches per core
HNS = NS // 2  # support cols per PE row-group
SPLIT = 1  # independent sub-scans per q-tile (1 measured best on HW)

_cache = {}


def _emit_body(nc, mybir, q_d, s_d, o_d, ones, rall, pools, rep=0,
               parts=3, split=SPLIT):
    DO_MM = parts & 1
    DO_EVAC = parts & 2
    f32 = mybir.dt.float32
    bf16 = mybir.dt.bfloat16
    fmax = mybir.AluOpType.max
    X = mybir.AxisListType.X
    qp, sp, pa, pb, scp, rp, finp = pools

    n_qt = NQ // 128  # 16 q-tiles per batch
    W = HNS // split  # positions per sub-scan

    for b in range(BPC):
        qt = qp.tile([128, NQ], bf16, tag="q", name=f"q{rep}_{b}")
        nc.sync.dma_start(out=qt[0:64, :], in_=q_d[b])
        nc.sync.dma_start(out=qt[64:128, :], in_=q_d[b])
        st = sp.tile([128, HNS], bf16, tag="s", name=f"s{rep}_{b}")
        nc.sync.dma_start(out=st[0:64, :], in_=s_d[b][:, 0:HNS])
        nc.sync.dma_start(out=st[64:128, :], in_=s_d[b][:, HNS:NS])

        pmax = rp.tile([128, n_qt, max(split, 1)], f32, tag="pmax",
                       name=f"pm{rep}_{b}")
        if not (DO_MM and DO_EVAC):
            nc.vector.memset(pmax[:], 0.0)
        scr = None
        if split == 0:
            # split=0: scans write a real stride-1 output; the row max is
            # extracted from the last scan position afterwards.
            scr = rp.tile([128, n_qt, HNS], bf16, tag="scr", name=f"sc{rep}_{b}")

        for i in range(n_qt):
            lhs0 = qt[0:64, i * 128 : (i + 1) * 128]
            lhs1 = qt[64:128, i * 128 : (i + 1) * 128]
            A = pa.tile([128, HNS], f32, tag="pa", name=f"A{rep}_{b}_{i}")
            Bt = pb.tile([128, HNS], f32, tag="pb", name=f"B{rep}_{b}_{i}")
            if not DO_MM:
                continue
            for j in range(2):
                sl = slice(j * 512, (j + 1) * 512)
                nc.tensor.matmul(A[:, sl], lhsT=lhs0, rhs=st[0:64, sl],
                                 start=True, stop=True)
                nc.tensor.matmul(Bt[:, sl], lhsT=lhs1, rhs=st[64:128, sl],
                                 start=True, stop=True, tile_position=(64, 0))
            if DO_EVAC:
                bh = scp.tile([128, HNS], bf16, tag="bh",
                              name=f"bh{rep}_{b}_{i}")
                nc.scalar.copy(out=bh[:], in_=Bt[:])
                for k in range(split):
                    ks = slice(k * W, (k + 1) * W)
                    nc.vector.tensor_tensor_scan(
                        out=pmax[:, i, k : k + 1].broadcast_to([128, W]),
                        data0=A[:, ks], data1=bh[:, ks], initial=-1e30,
                        op0=fmax, op1=fmax)

        pm2 = rp.tile([128, n_qt], f32, tag="pm2", name=f"pm2{rep}_{b}")
        nc.vector.reduce_max(pm2[:], pmax[:], axis=X)
        nc.vector.reduce_sum(rall[:, b : b + 1], pm2[:], axis=X)

    pf = pa.tile([128, HNS], f32, tag="pa", name=f"pf{rep}")
    nc.tensor.matmul(pf[0:1, 0:BPC], lhsT=ones[:], rhs=rall[:],
                     start=True, stop=True)
    ob = finp.tile([1, BPC], f32, tag="ob", name=f"ob{rep}")
    nc.scalar.mul(ob[:], pf[0:1, 0:BPC], 1.0 / NQ)
    nc.sync.dma_start(out=o_d[:], in_=ob[:])


def _build(loop_reps=None, parts=3, split=SPLIT, bufs_a=3, bufs_b=1):
    import concourse.bacc as bacc
    import concourse.mybir as mybir
    import concourse.tile as tile

    f32 = mybir.dt.float32
    bf16 = mybir.dt.bfloat16

    nc = bacc.Bacc("TRN2", target_bir_lowering=False, debug=False)
    q_d = nc.dram_tensor("q", [BPC, D, NQ], bf16, kind="ExternalInput").ap()
    s_d = nc.dram_tensor("s", [BPC, D, NS], bf16, kind="ExternalInput").ap()
    o_d = nc.dram_tensor("o", [1, BPC], f32, kind="ExternalOutput").ap()

    with tile.TileContext(nc) as tc:
        with (
            tc.tile_pool(name="qp", bufs=3) as qp,
            tc.tile_pool(name="sp", bufs=3) as sp,
            tc.tile_pool(name="pa", bufs=bufs_a, space="PSUM") as pa,
            tc.tile_pool(name="pb", bufs=bufs_b, space="PSUM") as pb,
            tc.tile_pool(name="scp", bufs=3) as scp,
            tc.tile_pool(name="rp", bufs=2) as rp,
            tc.tile_pool(name="fin", bufs=1) as finp,
        ):
            ones = finp.tile([128, 1], f32, tag="ones")
            nc.vector.memset(ones[:], 1.0)
            rall = finp.tile([128, BPC], f32, tag="rall")
            pools = (qp, sp, pa, pb, scp, rp, finp)

            if loop_reps is None:
                _emit_body(nc, mybir, q_d, s_d, o_d, ones, rall, pools,
                           parts=parts, split=split)
            else:
                with tc.For_i(0, loop_reps, 1):
                    _emit_body(nc, mybir, q_d, s_d, o_d, ones, rall, pools,
                               parts=parts, split=split)

    nc.compile()
    return nc


def _to_bf16(x):
    import ml_dtypes

    return np.ascontiguousarray(x, dtype=np.float32).astype(ml_dtypes.bfloat16)


def _prep(query_local, support_local):
    q = _to_bf16(query_local).reshape(N_CORES, BPC, D, NQ)
    s = _to_bf16(support_local).reshape(N_CORES, BPC, D, NS)
    return q, s


def kernel(query_local, support_local):
    from concourse.bass_utils import run_bass_kernel_spmd

    if "nc" not in _cache:
        _cache["nc"] = _build()
    nc = _cache["nc"]

    q, s = _prep(query_local, support_local)
    in_maps = [{"q": q[c], "s": s[c]} for c in range(N_CORES)]
    res = run_bass_kernel_spmd(nc, in_maps, list(range(N_CORES)))
    outs = [np.asarray(res.results[c]["o"]).reshape(BPC) for c in range(N_CORES)]
    return np.concatenate(outs, axis=0)
